# revision 18
# baseline (speedup 1.0000x reference)
"""Trainium2 Bass kernel for a binarized DownBlock:
  residual = x[:, :256]
  out = conv3x3(sign(x), sign(W))           # Cin=512 -> Cout=256, pad 1
  out = BatchNorm(train-mode batch stats) * gamma + beta
  out = clip(out + residual, -1, 1)

Sharding: data-parallel over batch, 8 images per core on 8 NeuronCores.
BN batch statistics (per-channel sum and sum-of-squares) are all-reduced
across the 8 cores (2KB AllReduce).

Device compute:
  - sign(x), sign(W) on the Scalar (ACT) engine -> bf16/fp8 (+/-1 exact)
  - conv as 9 shifted matmuls per output tile over a zero-halo input,
    contraction over Cin in chunks of 128 (bf16) or 256 (fp8 DoubleRow)
    partitions, accumulated in PSUM (fp32, exact integers)
  - PSUM drain + per-channel sums on DVE (keeps ACT free for signs)
  - residual kept from pass 1 as fp16 (saves an 8.4MB re-read)
  - epilogue split across ACT (affine) / DVE (residual add) / GpSimd
    (clamp) so the post-collective tail pipelines across engines
Host does only sharding / pure layout transforms (reshape/transpose).
"""

import numpy as np

import concourse.bass as bass
import concourse.bacc as bacc
import concourse.tile as tile
from concourse import mybir
from concourse.bass_utils import run_bass_kernel_spmd

F32 = mybir.dt.float32
F16 = mybir.dt.float16
BF16 = mybir.dt.bfloat16
FP8 = mybir.dt.float8e4
AF = mybir.ActivationFunctionType
ALU = mybir.AluOpType

N_CORES = 8
N_IMG = 8          # images per core
BN_EPS = 1e-5
MM_DTYPE = "fp8"   # "bf16" or "fp8" (DoubleRow)

# tap order: (0,0) first so the first matmul of each accumulation group
# covers the full PSUM zero-region (start=True overwrites everything).
TAPS = [(0, 0), (-1, -1), (-1, 0), (-1, 1), (0, -1), (0, 1), (1, -1), (1, 0), (1, 1)]


def build_program(n_img: int = N_IMG, n_cores: int = N_CORES,
                  debug_conv: bool = False,
                  use_collective: bool = True,
                  mm: str = MM_DTYPE) -> bass.Bass:
    nc = bacc.Bacc("TRN2", target_bir_lowering=False, debug=False,
                   enable_asserts=True, num_devices=n_cores)

    XD = BF16 if mm == "bf16" else FP8
    perf_mode = None if mm == "bf16" else mybir.MatmulPerfMode.DoubleRow
    kstep = 1 if mm == "bf16" else 2       # kc chunks consumed per matmul

    # x:  [img, kc, p, hw]   channel c = kc*128 + p, hw = y*32 + x
    x_d = nc.dram_tensor("x", [n_img, 4, 128, 1024], F32, kind="ExternalInput")
    # wt: [kc, p, tap, co]   pre-transposed on host (pure layout), fp32
    wt_d = nc.dram_tensor("wt", [4, 128, 9, 256], F32, kind="ExternalInput")
    # gb: [p, 4] = [gamma_mc0, gamma_mc1, beta_mc0, beta_mc1]
    gb_d = nc.dram_tensor("gb", [128, 4], F32, kind="ExternalInput")
    # y:  [img, mc, p, hw]   channel c = mc*128 + p
    y_d = nc.dram_tensor("y", [n_img, 2, 128, 1024], F32, kind="ExternalOutput")
    dbg_d = None
    if debug_conv:
        dbg_d = nc.dram_tensor("dbg", [2, n_img, 128, 1024], F32,
                               kind="ExternalOutput")

    inv_n = 1.0 / float(n_cores * n_img * 1024)

    with tile.TileContext(nc) as tc:
        with (
            tc.tile_pool(name="const", bufs=1) as constp,
            tc.tile_pool(name="wstage", bufs=2) as wstagep,
            tc.tile_pool(name="xs", bufs=3) as xsp,
            tc.tile_pool(name="xb", bufs=1) as xbp,
            tc.tile_pool(name="conv", bufs=1) as convp,
            tc.tile_pool(name="res", bufs=3) as resp,
            tc.tile_pool(name="ob", bufs=3) as obp,
            tc.tile_pool(name="psum", bufs=6, space="PSUM") as psump,
            tc.tile_pool(name="dram", bufs=1, space="DRAM") as dramp,
        ):
            # ---- weights: DMA fp32 per kc chunk, sign -> XD
            wT = constp.tile([128, 4, 9, 256], XD)
            for kc in range(4):
                w_st = wstagep.tile([128, 2304], F32, tag="wst", name="w_st")
                nc.sync.dma_start(
                    w_st[:].rearrange("p (t c) -> p t c", c=256), wt_d[kc])
                nc.scalar.activation(
                    wT[:, kc], w_st[:].rearrange("p (t c) -> p t c", c=256),
                    AF.Sign)

            gb_sb = constp.tile([128, 4], F32)
            nc.sync.dma_start(gb_sb[:], gb_d[:])

            conv_sb = convp.tile([128, 2, n_img, 1024], F32)
            sum_acc = constp.tile([128, 2, 2 * n_img], F32)
            sq_acc = constp.tile([128, 2, n_img], F32)
            junk = constp.tile([128, 1024], F32)

            # ---- pass 1: conv + local stats
            # binarized input with a zero halo: [p, kc, 34, 34]; every tap
            # then yields a full contiguous [128, 512] PSUM tile.
            xpads = [xbp.tile([128, 4, 34, 34], XD, name=f"xpad{j}")
                     for j in range(2)]
            for xp in xpads:
                nc.gpsimd.memset(xp[:], 0.0)

            for i in range(n_img):
                xp = xpads[i % 2]
                for kc in range(4):
                    xs_t = xsp.tile([128, 1024], F32, tag="xs", name="xs_t")
                    nc.sync.dma_start(xs_t[:], x_d[i, kc])
                    nc.scalar.activation(
                        xp[:, kc, 1:33, 1:33],
                        xs_t[:].rearrange("p (y x) -> p y x", x=32), AF.Sign)

                for mc in range(2):
                    pts = [psump.tile([128, 512], F32, tag="pt",
                                      name=f"pt_{i}_{mc}_{sp}")
                           for sp in range(2)]
                    for ti, (dh, dw) in enumerate(TAPS):
                        tw = (dh + 1) * 3 + (dw + 1)  # weight tap (kh*3+kw)
                        for kc in range(0, 4, kstep):
                            if kstep == 1:
                                w_ap = wT[:, kc, tw, mc * 128:(mc + 1) * 128]
                            else:
                                w_ap = wT[:, kc:kc + 2, tw,
                                          mc * 128:(mc + 1) * 128]
                            for sp in range(2):
                                r0 = sp * 16
                                if kstep == 1:
                                    rhs_ap = xp[:, kc,
                                                r0 + dh + 1:r0 + dh + 17,
                                                dw + 1:dw + 33]
                                else:
                                    rhs_ap = xp[:, kc:kc + 2,
                                                r0 + dh + 1:r0 + dh + 17,
                                                dw + 1:dw + 33]
                                nc.tensor.matmul(
                                    pts[sp][:], w_ap, rhs_ap,
                                    start=(ti == 0 and kc == 0),
                                    stop=(ti == len(TAPS) - 1
                                          and kc + kstep >= 4),
                                    perf_mode=perf_mode,
                                )
                    # drain + per-channel sums on DVE
                    for sp in range(2):
                        u = i * 2 + sp
                        nc.vector.tensor_scalar(
                            conv_sb[:, mc, i, 512 * sp:512 * (sp + 1)],
                            pts[sp][:], 0.0, None, ALU.add, ALU.add,
                            accum_out=sum_acc[:, mc, u:u + 1])
                    # sum of squares for the whole image row (ACT Square)
                    nc.scalar.activation(
                        junk[:], conv_sb[:, mc, i], AF.Square,
                        accum_out=sq_acc[:, mc, i:i + 1])

            if dbg_d is not None:
                nc.sync.dma_start(dbg_d[:].rearrange("m i p hw -> p m i hw"),
                                  conv_sb[:])

            # ---- stats reduce + AllReduce across cores
            st_l = constp.tile([128, 4], F32)
            nc.vector.tensor_reduce(st_l[:, 0:2], sum_acc[:],
                                    mybir.AxisListType.X, ALU.add)
            nc.vector.tensor_reduce(st_l[:, 2:4], sq_acc[:],
                                    mybir.AxisListType.X, ALU.add)

            st_g = constp.tile([128, 4], F32)
            if use_collective:
                cc_in = dramp.tile([128, 4], F32, name="cc_in")
                cc_out = dramp.tile([128, 4], F32, addr_space="Shared",
                                    name="cc_out")
                nc.sync.dma_start(cc_in[:], st_l[:])
                nc.gpsimd.collective_compute(
                    "AllReduce", ALU.add,
                    replica_groups=[list(range(n_cores))],
                    ins=[cc_in.opt()], outs=[cc_out.opt()])
                nc.sync.dma_start(st_g[:], cc_out[:])
            else:
                # timing-only build (TimelineSim can't model collectives)
                nc.vector.tensor_copy(st_g[:], st_l[:])

            # ---- finalize BN affine: scale = gamma*rsqrt(var+eps),
            #      shift = beta - mean*scale
            mean_t = constp.tile([128, 2], F32)
            ex2_t = constp.tile([128, 2], F32)
            var_t = constp.tile([128, 2], F32)
            sd_t = constp.tile([128, 2], F32)
            inv_t = constp.tile([128, 2], F32)
            scale_t = constp.tile([128, 2], F32)
            shift_t = constp.tile([128, 2], F32)

            nc.vector.tensor_scalar(mean_t[:], st_g[:, 0:2], inv_n, None,
                                    ALU.mult)
            nc.vector.tensor_scalar(ex2_t[:], st_g[:, 2:4], inv_n, None,
                                    ALU.mult)
            nc.vector.tensor_tensor(var_t[:], mean_t[:], mean_t[:], ALU.mult)
            nc.vector.tensor_tensor(var_t[:], ex2_t[:], var_t[:], ALU.subtract)
            eps_t = constp.tile([128, 1], F32)
            nc.vector.memset(eps_t[:], BN_EPS)
            nc.scalar.activation(sd_t[:], var_t[:], AF.Sqrt, bias=eps_t[:])
            nc.vector.reciprocal(inv_t[:], sd_t[:])
            nc.vector.tensor_tensor(scale_t[:], gb_sb[:, 0:2], inv_t[:],
                                    ALU.mult)
            nc.vector.tensor_tensor(shift_t[:], mean_t[:], scale_t[:],
                                    ALU.mult)
            nc.vector.tensor_tensor(shift_t[:], gb_sb[:, 2:4], shift_t[:],
                                    ALU.subtract)

            # ---- pass 2: affine (ACT) + residual add (DVE) + clamp (GpSimd)
            for i in range(n_img):
                res_t = resp.tile([128, 2, 1024], F32, tag="res", name="res_t")
                nc.sync.dma_start(res_t[:],
                                  x_d[i, 0:2].rearrange("k p hw -> p k hw"))
                for mc in range(2):
                    ob_t = obp.tile([128, 1024], F32, tag="ob", name="ob_t")
                    nc.scalar.activation(ob_t[:], conv_sb[:, mc, i],
                                         AF.Identity,
                                         bias=shift_t[:, mc:mc + 1],
                                         scale=scale_t[:, mc:mc + 1])
                    nc.vector.tensor_tensor(ob_t[:], ob_t[:],
                                            res_t[:, mc], ALU.add)
                    nc.gpsimd.tensor_scalar(ob_t[:], ob_t[:], 1.0, -1.0,
                                            ALU.min, ALU.max)
                    nc.sync.dma_start(y_d[i, mc], ob_t[:])

    nc.compile()
    return nc


def _prep_inputs(x, W, gamma, beta):
    """Pure layout transforms + batch sharding (no arithmetic)."""
    x = np.ascontiguousarray(x, dtype=np.float32)
    W = np.ascontiguousarray(W, dtype=np.float32)
    # [core, img, kc, p, hw]
    xs = x.reshape(N_CORES, N_IMG, 4, 128, 1024)
    # W [co, ci, 3, 3] -> wt[kc, p, tap, co]
    wt = np.ascontiguousarray(
        W.reshape(256, 4, 128, 9).transpose(1, 2, 3, 0))
    g2 = np.asarray(gamma, np.float32).reshape(2, 128).T   # [p, mc]
    b2 = np.asarray(beta, np.float32).reshape(2, 128).T
    gb = np.ascontiguousarray(np.concatenate([g2, b2], axis=1))  # [128, 4]
    return xs, wt, gb


_NC_CACHE = {}


def kernel(x, W, gamma, beta):
    key = "prog"
    if key not in _NC_CACHE:
        _NC_CACHE[key] = build_program()
    nc = _NC_CACHE[key]

    xs, wt, gb = _prep_inputs(x, W, gamma, beta)
    in_maps = [{"x": np.ascontiguousarray(xs[c]), "wt": wt, "gb": gb}
               for c in range(N_CORES)]
    res = run_bass_kernel_spmd(nc, in_maps, list(range(N_CORES)))
    y = np.concatenate(
        [np.asarray(res.results[c]["y"]).reshape(N_IMG, 256, 32, 32)
         for c in range(N_CORES)], axis=0)
    return y
